# revision 14
# baseline (speedup 1.0000x reference)
"""Householder reflection kernel for Trainium2, data-parallel over 8 NeuronCores.

out = z - 2 * v * (v.z) / (v.v), rowwise over [8192, 2048] f32.

Sharding: batch dim split 8 ways (1024 rows/core); no cross-core communication.
HBM-bandwidth bound (24 MiB of traffic per core). Schedule per core:
  - all input tiles stay resident in SBUF (16 MiB inputs -> 128 KiB/partition)
  - v loads issue on the Sync (SP) HWDGE ring, z loads on the Scalar (ACT)
    HWDGE ring: two FIFO rings overlap each other's inter-DMA handoff bubbles
  - small head tiles (1 block) so compute starts early, 2-block (2 MiB) tiles
    for the bulk
  - per 128-row block: DVE scalar_tensor_tensor computes v*z with rowsum
    accum (vz), ACT activation(Square) computes rowsum(v^2) (nsq), DVE
    reciprocal + tiny STT give s = -2*vz/nsq, and one fused DVE STT computes
    the result IN PLACE into the z tile (no separate output tile)
  - stores (per macro tile, SWDGE/gpsimd ring) are gated on the last load so
    they never steal packet-round-robin turns from the load streams
"""

from contextlib import ExitStack

import numpy as np

import concourse.bacc as bacc
import concourse.bass as bass
import concourse.tile as tile
from concourse import mybir
from concourse.bass_utils import run_bass_kernel_spmd

N_CORES = 8
B, L = 8192, 2048
RPC = B // N_CORES   # rows per core
P = 128              # SBUF partitions
TILE_BLOCKS = [1, 1, 2, 2, 2]   # macro-tile sizes in 128-row blocks
assert sum(TILE_BLOCKS) * P == RPC

_NC = None


def build_nc() -> bass.Bass:
    nc = bacc.Bacc("TRN2")
    f32 = mybir.dt.float32
    v = nc.declare_dram_parameter("v", [RPC, L], f32, isOutput=False)
    z = nc.declare_dram_parameter("z", [RPC, L], f32, isOutput=False)
    out = nc.declare_dram_parameter("out", [RPC, L], f32, isOutput=True)

    n_small = sum(1 for b in TILE_BLOCKS if b == 1)
    n_big = len(TILE_BLOCKS) - n_small

    with tile.TileContext(nc) as tc, ExitStack() as ctx:
        vs_pool = ctx.enter_context(tc.tile_pool(name="vs", bufs=max(n_small, 1)))
        vb_pool = ctx.enter_context(tc.tile_pool(name="vb", bufs=max(n_big, 1)))
        zs_pool = ctx.enter_context(tc.tile_pool(name="zs", bufs=max(n_small, 1)))
        zb_pool = ctx.enter_context(tc.tile_pool(name="zb", bufs=max(n_big, 1)))
        spool = ctx.enter_context(tc.tile_pool(name="sp", bufs=1))
        stats = ctx.enter_context(tc.tile_pool(name="st", bufs=6))

        # write-only sinks for the reduction ops' full outputs (never read)
        prod_sink = spool.tile([P, L], f32, tag="prod")
        sq_sink = spool.tile([P, L], f32, tag="sq")

        # ---- issue every load up front: v on SP ring, z on ACT ring ----
        work = []  # (r0, nb, vt, zt)
        r0 = 0
        last_load = None
        with tc.high_priority():
            for nb in TILE_BLOCKS:
                rows = P * nb
                src_v = v[r0 : r0 + rows].rearrange("(a p) m -> p a m", p=P)
                src_z = z[r0 : r0 + rows].rearrange("(a p) m -> p a m", p=P)

                vpool = vs_pool if nb == 1 else vb_pool
                zpool = zs_pool if nb == 1 else zb_pool
                vt = vpool.tile([P, nb, L], f32)
                nc.sync.dma_start(vt[:], src_v)
                zt = zpool.tile([P, nb, L], f32)
                last_load = nc.scalar.dma_start(zt[:], src_z)
                work.append((r0, nb, vt, zt))
                r0 += rows

        # ---- compute per 128-row block, result in place into zt ----
        for r0, nb, vt, zt in work:
            for a in range(nb):
                va = vt[:, a, :]
                za = zt[:, a, :]

                vz = stats.tile([P, 1], f32, tag="vz")
                nc.vector.scalar_tensor_tensor(
                    out=prod_sink[:], in0=va, scalar=1.0, in1=za,
                    op0=mybir.AluOpType.bypass, op1=mybir.AluOpType.mult,
                    accum_out=vz[:],
                )

                nsq = stats.tile([P, 1], f32, tag="nsq")
                nc.scalar.activation(
                    out=sq_sink[:], in_=va,
                    func=mybir.ActivationFunctionType.Square,
                    accum_out=nsq[:],
                )

                r = stats.tile([P, 1], f32, tag="r")
                nc.vector.reciprocal(r[:], nsq[:])
                s = stats.tile([P, 1], f32, tag="s")
                nc.vector.scalar_tensor_tensor(
                    out=s[:], in0=vz[:], scalar=-2.0, in1=r[:],
                    op0=mybir.AluOpType.mult, op1=mybir.AluOpType.mult,
                )

                nc.vector.scalar_tensor_tensor(
                    out=za, in0=va, scalar=s[:], in1=za,
                    op0=mybir.AluOpType.mult, op1=mybir.AluOpType.add,
                )

            # ---- store the whole macro tile (SWDGE), after all loads ----
            dst_o = out[r0 : r0 + P * nb].rearrange("(a p) m -> p a m", p=P)
            st_dma = nc.gpsimd.dma_start(dst_o, zt[:])
            tile.add_dep_helper(
                st_dma.ins, last_load.ins, sync=True,
                reason="store after all loads",
            )

    nc.compile()  # bacc: split sync waits, alloc regs, fuse nops
    return nc


def _get_nc() -> bass.Bass:
    global _NC
    if _NC is None:
        _NC = build_nc()
    return _NC


def _in_maps(v: np.ndarray, z: np.ndarray) -> list[dict]:
    v = np.ascontiguousarray(np.asarray(v, dtype=np.float32))
    z = np.ascontiguousarray(np.asarray(z, dtype=np.float32))
    return [
        {"v": v[i * RPC : (i + 1) * RPC], "z": z[i * RPC : (i + 1) * RPC]}
        for i in range(N_CORES)
    ]


def run_spmd(v: np.ndarray, z: np.ndarray, **kwargs):
    """Run on all 8 cores; returns BassKernelResults (kwargs e.g. trace=True)."""
    return run_bass_kernel_spmd(_get_nc(), _in_maps(v, z), list(range(N_CORES)), **kwargs)


def kernel(v: np.ndarray, z: np.ndarray) -> np.ndarray:
    res = run_spmd(v, z)
    return np.concatenate([res.results[i]["out"] for i in range(N_CORES)], axis=0)


# revision 16
# speedup vs baseline: 1.1221x; 1.1221x over previous
"""Householder reflection kernel for Trainium2, data-parallel over 8 NeuronCores.

out = z - 2 * v * (v.z) / (v.v), rowwise over [8192, 2048] f32.

Sharding: batch dim split 8 ways (1024 rows/core); no cross-core communication.
HBM-bandwidth bound (24 MiB of traffic per core). Schedule per core:
  - all input tiles stay resident in SBUF (16 MiB inputs -> 128 KiB/partition)
  - v loads issue on the Sync (SP) HWDGE ring, z loads on the Scalar (ACT)
    HWDGE ring: two FIFO rings overlap each other's inter-DMA handoff bubbles
  - small head tiles (1 block) so compute starts early, 2-block (2 MiB) tiles
    for the bulk
  - per 128-row block: DVE scalar_tensor_tensor computes v*z with rowsum
    accum (vz), ACT activation(Square) computes rowsum(v^2) (nsq), DVE
    reciprocal + tiny STT give s = -2*vz/nsq, and one fused DVE STT computes
    the result IN PLACE into the z tile (no separate output tile)
  - stores (per macro tile, SWDGE/gpsimd ring) are gated on the last load so
    they never steal packet-round-robin turns from the load streams
"""

from contextlib import ExitStack

import numpy as np

import concourse.bacc as bacc
import concourse.bass as bass
import concourse.tile as tile
from concourse import mybir
from concourse.bass_utils import run_bass_kernel_spmd

N_CORES = 8
B, L = 8192, 2048
RPC = B // N_CORES   # rows per core
P = 128              # SBUF partitions
TILE_BLOCKS = [2, 2, 2, 1, 1]   # macro-tile sizes in 128-row blocks
assert sum(TILE_BLOCKS) * P == RPC

_NC = None


def build_nc() -> bass.Bass:
    nc = bacc.Bacc("TRN2")
    f32 = mybir.dt.float32
    v = nc.declare_dram_parameter("v", [RPC, L], f32, isOutput=False)
    z = nc.declare_dram_parameter("z", [RPC, L], f32, isOutput=False)
    out = nc.declare_dram_parameter("out", [RPC, L], f32, isOutput=True)

    n_small = sum(1 for b in TILE_BLOCKS if b == 1)
    n_big = len(TILE_BLOCKS) - n_small

    with tile.TileContext(nc) as tc, ExitStack() as ctx:
        vs_pool = ctx.enter_context(tc.tile_pool(name="vs", bufs=max(n_small, 1)))
        vb_pool = ctx.enter_context(tc.tile_pool(name="vb", bufs=max(n_big, 1)))
        zs_pool = ctx.enter_context(tc.tile_pool(name="zs", bufs=max(n_small, 1)))
        zb_pool = ctx.enter_context(tc.tile_pool(name="zb", bufs=max(n_big, 1)))
        spool = ctx.enter_context(tc.tile_pool(name="sp", bufs=1))
        stats = ctx.enter_context(tc.tile_pool(name="st", bufs=6))

        # write-only sinks for the reduction ops' full outputs (never read)
        prod_sink = spool.tile([P, L], f32, tag="prod")
        sq_sink = spool.tile([P, L], f32, tag="sq")

        # ---- issue every load up front: v on SP ring, z on ACT ring ----
        work = []  # (r0, nb, vt, zt)
        r0 = 0
        last_load = None
        with tc.high_priority():
            for nb in TILE_BLOCKS:
                rows = P * nb
                src_v = v[r0 : r0 + rows].rearrange("(a p) m -> p a m", p=P)
                src_z = z[r0 : r0 + rows].rearrange("(a p) m -> p a m", p=P)

                vpool = vs_pool if nb == 1 else vb_pool
                zpool = zs_pool if nb == 1 else zb_pool
                vt = vpool.tile([P, nb, L], f32)
                nc.sync.dma_start(vt[:], src_v)
                zt = zpool.tile([P, nb, L], f32)
                last_load = nc.scalar.dma_start(zt[:], src_z)
                work.append((r0, nb, vt, zt))
                r0 += rows

        # ---- compute per 128-row block, result in place into zt ----
        for r0, nb, vt, zt in work:
            for a in range(nb):
                va = vt[:, a, :]
                za = zt[:, a, :]

                vz = stats.tile([P, 1], f32, tag="vz")
                nc.vector.scalar_tensor_tensor(
                    out=prod_sink[:], in0=va, scalar=1.0, in1=za,
                    op0=mybir.AluOpType.bypass, op1=mybir.AluOpType.mult,
                    accum_out=vz[:],
                )

                nsq = stats.tile([P, 1], f32, tag="nsq")
                nc.scalar.activation(
                    out=sq_sink[:], in_=va,
                    func=mybir.ActivationFunctionType.Square,
                    accum_out=nsq[:],
                )

                r = stats.tile([P, 1], f32, tag="r")
                nc.vector.reciprocal(r[:], nsq[:])
                s = stats.tile([P, 1], f32, tag="s")
                nc.vector.scalar_tensor_tensor(
                    out=s[:], in0=vz[:], scalar=-2.0, in1=r[:],
                    op0=mybir.AluOpType.mult, op1=mybir.AluOpType.mult,
                )

                nc.vector.scalar_tensor_tensor(
                    out=za, in0=va, scalar=s[:], in1=za,
                    op0=mybir.AluOpType.mult, op1=mybir.AluOpType.add,
                )

            # ---- store the whole macro tile (SWDGE ring, overlaps loads) ----
            dst_o = out[r0 : r0 + P * nb].rearrange("(a p) m -> p a m", p=P)
            nc.gpsimd.dma_start(dst_o, zt[:])

    nc.compile()  # bacc: split sync waits, alloc regs, fuse nops
    return nc


def _get_nc() -> bass.Bass:
    global _NC
    if _NC is None:
        _NC = build_nc()
    return _NC


def _in_maps(v: np.ndarray, z: np.ndarray) -> list[dict]:
    v = np.ascontiguousarray(np.asarray(v, dtype=np.float32))
    z = np.ascontiguousarray(np.asarray(z, dtype=np.float32))
    return [
        {"v": v[i * RPC : (i + 1) * RPC], "z": z[i * RPC : (i + 1) * RPC]}
        for i in range(N_CORES)
    ]


def run_spmd(v: np.ndarray, z: np.ndarray, **kwargs):
    """Run on all 8 cores; returns BassKernelResults (kwargs e.g. trace=True)."""
    return run_bass_kernel_spmd(_get_nc(), _in_maps(v, z), list(range(N_CORES)), **kwargs)


def kernel(v: np.ndarray, z: np.ndarray) -> np.ndarray:
    res = run_spmd(v, z)
    return np.concatenate([res.results[i]["out"] for i in range(N_CORES)], axis=0)


# revision 19
# speedup vs baseline: 1.1621x; 1.0356x over previous
"""Householder reflection kernel for Trainium2, data-parallel over 8 NeuronCores.

out = z - 2 * v * (v.z) / (v.v), rowwise over [8192, 2048] f32.

Sharding: batch dim split 8 ways (1024 rows/core); no cross-core communication.
HBM-bandwidth bound (24 MiB of traffic per core). Schedule per core:
  - all input tiles stay resident in SBUF (16 MiB inputs -> 128 KiB/partition)
  - v loads issue on the Sync (SP) HWDGE ring, z loads on the Scalar (ACT)
    HWDGE ring: two FIFO rings overlap each other's inter-DMA handoff bubbles
  - small head tiles (1 block) so compute starts early, 2-block (2 MiB) tiles
    for the bulk
  - per 128-row block: DVE scalar_tensor_tensor computes v*z with rowsum
    accum (vz), ACT activation(Square) computes rowsum(v^2) (nsq), DVE
    reciprocal + tiny STT give s = -2*vz/nsq, and one fused DVE STT computes
    the result IN PLACE into the z tile (no separate output tile)
  - stores (per macro tile, SWDGE/gpsimd ring) are gated on the last load so
    they never steal packet-round-robin turns from the load streams
"""

from contextlib import ExitStack

import numpy as np

import concourse.bacc as bacc
import concourse.bass as bass
import concourse.tile as tile
from concourse import mybir
from concourse.bass_utils import run_bass_kernel_spmd

N_CORES = 8
B, L = 8192, 2048
RPC = B // N_CORES   # rows per core
P = 128              # SBUF partitions
TILE_BLOCKS = [1, 1, 2, 2, 1, 1]   # macro-tile sizes in 128-row blocks
assert sum(TILE_BLOCKS) * P == RPC

_NC = None


def build_nc() -> bass.Bass:
    nc = bacc.Bacc("TRN2")
    f32 = mybir.dt.float32
    v = nc.declare_dram_parameter("v", [RPC, L], f32, isOutput=False)
    z = nc.declare_dram_parameter("z", [RPC, L], f32, isOutput=False)
    out = nc.declare_dram_parameter("out", [RPC, L], f32, isOutput=True)

    n_small = sum(1 for b in TILE_BLOCKS if b == 1)
    n_big = len(TILE_BLOCKS) - n_small

    with tile.TileContext(nc) as tc, ExitStack() as ctx:
        vs_pool = ctx.enter_context(tc.tile_pool(name="vs", bufs=max(n_small, 1)))
        vb_pool = ctx.enter_context(tc.tile_pool(name="vb", bufs=max(n_big, 1)))
        zs_pool = ctx.enter_context(tc.tile_pool(name="zs", bufs=max(n_small, 1)))
        zb_pool = ctx.enter_context(tc.tile_pool(name="zb", bufs=max(n_big, 1)))
        spool = ctx.enter_context(tc.tile_pool(name="sp", bufs=1))
        stats = ctx.enter_context(tc.tile_pool(name="st", bufs=6))

        # write-only sinks for the reduction ops' full outputs (never read)
        prod_sink = spool.tile([P, L], f32, tag="prod")
        sq_sink = spool.tile([P, L], f32, tag="sq")

        # ---- issue every load up front, pair-balanced across the two HWDGE
        # rings (SP and ACT): macro k puts v on one ring and z on the other,
        # alternating, so each (v_k, z_k) pair completes as early as possible
        work = []  # (r0, nb, vt, zt)
        r0 = 0
        with tc.high_priority():
            for k, nb in enumerate(TILE_BLOCKS):
                rows = P * nb
                src_v = v[r0 : r0 + rows].rearrange("(a p) m -> p a m", p=P)
                src_z = z[r0 : r0 + rows].rearrange("(a p) m -> p a m", p=P)

                vpool = vs_pool if nb == 1 else vb_pool
                zpool = zs_pool if nb == 1 else zb_pool
                vt = vpool.tile([P, nb, L], f32)
                zt = zpool.tile([P, nb, L], f32)
                eng_v, eng_z = (nc.sync, nc.scalar) if k % 2 == 0 else (nc.scalar, nc.sync)
                eng_v.dma_start(vt[:], src_v)
                eng_z.dma_start(zt[:], src_z)
                work.append((r0, nb, vt, zt))
                r0 += rows

        # ---- compute per 128-row block, result in place into zt ----
        final_stts = []  # per-block final STT instructions, for order pinning
        for r0, nb, vt, zt in work:
            for a in range(nb):
                va = vt[:, a, :]
                za = zt[:, a, :]
                b = len(final_stts)  # global block index

                vz = stats.tile([P, 1], f32, tag="vz")
                sttacc = nc.vector.scalar_tensor_tensor(
                    out=prod_sink[:], in0=va, scalar=1.0, in1=za,
                    op0=mybir.AluOpType.bypass, op1=mybir.AluOpType.mult,
                    accum_out=vz[:],
                )
                # Pin DVE order: don't let the scheduler hoist all the
                # reductions ahead of earlier blocks' output STTs (that
                # defers every store to the kernel tail).
                if b >= 2:
                    tile.add_dep_helper(
                        sttacc.ins, final_stts[b - 2].ins, sync=False,
                        reason="DVE order: reduce(b) after out-STT(b-2)",
                    )

                nsq = stats.tile([P, 1], f32, tag="nsq")
                nc.scalar.activation(
                    out=sq_sink[:], in_=va,
                    func=mybir.ActivationFunctionType.Square,
                    accum_out=nsq[:],
                )

                r = stats.tile([P, 1], f32, tag="r")
                nc.vector.reciprocal(r[:], nsq[:])
                s = stats.tile([P, 1], f32, tag="s")
                nc.vector.scalar_tensor_tensor(
                    out=s[:], in0=vz[:], scalar=-2.0, in1=r[:],
                    op0=mybir.AluOpType.mult, op1=mybir.AluOpType.mult,
                )

                final_stts.append(nc.vector.scalar_tensor_tensor(
                    out=za, in0=va, scalar=s[:], in1=za,
                    op0=mybir.AluOpType.mult, op1=mybir.AluOpType.add,
                ))

            # ---- store the whole macro tile (SWDGE ring, overlaps loads) ----
            dst_o = out[r0 : r0 + P * nb].rearrange("(a p) m -> p a m", p=P)
            nc.gpsimd.dma_start(dst_o, zt[:])

    nc.compile()  # bacc: split sync waits, alloc regs, fuse nops
    return nc


def _get_nc() -> bass.Bass:
    global _NC
    if _NC is None:
        _NC = build_nc()
    return _NC


def _in_maps(v: np.ndarray, z: np.ndarray) -> list[dict]:
    v = np.ascontiguousarray(np.asarray(v, dtype=np.float32))
    z = np.ascontiguousarray(np.asarray(z, dtype=np.float32))
    return [
        {"v": v[i * RPC : (i + 1) * RPC], "z": z[i * RPC : (i + 1) * RPC]}
        for i in range(N_CORES)
    ]


def run_spmd(v: np.ndarray, z: np.ndarray, **kwargs):
    """Run on all 8 cores; returns BassKernelResults (kwargs e.g. trace=True)."""
    return run_bass_kernel_spmd(_get_nc(), _in_maps(v, z), list(range(N_CORES)), **kwargs)


def kernel(v: np.ndarray, z: np.ndarray) -> np.ndarray:
    res = run_spmd(v, z)
    return np.concatenate([res.results[i]["out"] for i in range(N_CORES)], axis=0)


# revision 22
# speedup vs baseline: 1.2263x; 1.0553x over previous
"""Householder reflection kernel for Trainium2, data-parallel over 8 NeuronCores.

out = z - 2 * v * (v.z) / (v.v), rowwise over [8192, 2048] f32.

Sharding: batch dim split 8 ways (1024 rows/core); no cross-core communication.
HBM-bandwidth bound (24 MiB of traffic per core). Schedule per core:
  - all input tiles stay resident in SBUF (16 MiB inputs -> 128 KiB/partition)
  - v loads issue on the Sync (SP) HWDGE ring, z loads on the Scalar (ACT)
    HWDGE ring: two FIFO rings overlap each other's inter-DMA handoff bubbles
  - small head tiles (1 block) so compute starts early, 2-block (2 MiB) tiles
    for the bulk
  - per 128-row block: DVE scalar_tensor_tensor computes v*z with rowsum
    accum (vz), ACT activation(Square) computes rowsum(v^2) (nsq), DVE
    reciprocal + tiny STT give s = -2*vz/nsq, and one fused DVE STT computes
    the result IN PLACE into the z tile (no separate output tile)
  - stores (per macro tile, SWDGE/gpsimd ring) are gated on the last load so
    they never steal packet-round-robin turns from the load streams
"""

from contextlib import ExitStack

import numpy as np

import concourse.bacc as bacc
import concourse.bass as bass
import concourse.tile as tile
from concourse import mybir
from concourse.bass_utils import run_bass_kernel_spmd

N_CORES = 8
B, L = 8192, 2048
RPC = B // N_CORES   # rows per core
P = 128              # SBUF partitions
TILE_BLOCKS = [1, 1, 2, 2, 1, 1]   # macro-tile sizes in 128-row blocks
assert sum(TILE_BLOCKS) * P == RPC

_NC = None


def build_nc() -> bass.Bass:
    nc = bacc.Bacc("TRN2")
    f32 = mybir.dt.float32
    v = nc.declare_dram_parameter("v", [RPC, L], f32, isOutput=False)
    z = nc.declare_dram_parameter("z", [RPC, L], f32, isOutput=False)
    out = nc.declare_dram_parameter("out", [RPC, L], f32, isOutput=True)

    n_small = sum(1 for b in TILE_BLOCKS if b == 1)
    n_big = len(TILE_BLOCKS) - n_small

    with tile.TileContext(nc) as tc, ExitStack() as ctx:
        vs_pool = ctx.enter_context(tc.tile_pool(name="vs", bufs=max(n_small, 1)))
        vb_pool = ctx.enter_context(tc.tile_pool(name="vb", bufs=max(n_big, 1)))
        zs_pool = ctx.enter_context(tc.tile_pool(name="zs", bufs=max(n_small, 1)))
        zb_pool = ctx.enter_context(tc.tile_pool(name="zb", bufs=max(n_big, 1)))
        spool = ctx.enter_context(tc.tile_pool(name="sp", bufs=1))
        stats = ctx.enter_context(tc.tile_pool(name="st", bufs=6))

        # write-only sinks for the reduction ops' full outputs (never read)
        prod_sink = spool.tile([P, L], f32, tag="prod")
        sq_sink = spool.tile([P, L], f32, tag="sq")

        # ---- issue every load up front, pair-balanced across the two HWDGE
        # rings (SP and ACT): macro k puts v on one ring and z on the other,
        # alternating, so each (v_k, z_k) pair completes as early as possible
        work = []  # (r0, nb, vt, zt)
        r0 = 0
        with tc.high_priority():
            for k, nb in enumerate(TILE_BLOCKS):
                rows = P * nb
                src_v = v[r0 : r0 + rows].rearrange("(a p) m -> p a m", p=P)
                src_z = z[r0 : r0 + rows].rearrange("(a p) m -> p a m", p=P)

                vpool = vs_pool if nb == 1 else vb_pool
                zpool = zs_pool if nb == 1 else zb_pool
                vt = vpool.tile([P, nb, L], f32)
                zt = zpool.tile([P, nb, L], f32)
                eng_v, eng_z = (nc.sync, nc.scalar) if k % 2 == 0 else (nc.scalar, nc.sync)
                eng_v.dma_start(vt[:], src_v)
                eng_z.dma_start(zt[:], src_z)
                work.append((r0, nb, vt, zt))
                r0 += rows

        # ---- compute per 128-row block, result in place into zt ----
        final_stts = []  # per-block final STT instructions, for order pinning
        for r0, nb, vt, zt in work:
            for a in range(nb):
                va = vt[:, a, :]
                za = zt[:, a, :]
                b = len(final_stts)  # global block index

                vz = stats.tile([P, 1], f32, tag="vz")
                sttacc = nc.vector.scalar_tensor_tensor(
                    out=prod_sink[:], in0=va, scalar=1.0, in1=za,
                    op0=mybir.AluOpType.bypass, op1=mybir.AluOpType.mult,
                    accum_out=vz[:],
                )
                # Pin DVE order: don't let the scheduler hoist all the
                # reductions ahead of earlier blocks' output STTs (that
                # defers every store to the kernel tail).
                if b >= 2:
                    tile.add_dep_helper(
                        sttacc.ins, final_stts[b - 2].ins, sync=False,
                        reason="DVE order: reduce(b) after out-STT(b-2)",
                    )

                nsq = stats.tile([P, 1], f32, tag="nsq")
                nc.scalar.activation(
                    out=sq_sink[:], in_=va,
                    func=mybir.ActivationFunctionType.Square,
                    accum_out=nsq[:],
                )

                r = stats.tile([P, 1], f32, tag="r")
                nc.vector.reciprocal(r[:], nsq[:])
                s = stats.tile([P, 1], f32, tag="s")
                nc.vector.scalar_tensor_tensor(
                    out=s[:], in0=vz[:], scalar=-2.0, in1=r[:],
                    op0=mybir.AluOpType.mult, op1=mybir.AluOpType.mult,
                )

                final_stts.append(nc.vector.scalar_tensor_tensor(
                    out=za, in0=va, scalar=s[:], in1=za,
                    op0=mybir.AluOpType.mult, op1=mybir.AluOpType.add,
                ))

            # ---- store the whole macro tile on the SP HWDGE ring; ring order
            # puts every store after every load, so stores never head-of-line
            # block loads, and HWDGE stores avoid SWDGE's Q7 descriptor costs
            dst_o = out[r0 : r0 + P * nb].rearrange("(a p) m -> p a m", p=P)
            nc.sync.dma_start(dst_o, zt[:])

    nc.compile()  # bacc: split sync waits, alloc regs, fuse nops
    return nc


def _get_nc() -> bass.Bass:
    global _NC
    if _NC is None:
        _NC = build_nc()
    return _NC


def _in_maps(v: np.ndarray, z: np.ndarray) -> list[dict]:
    v = np.ascontiguousarray(np.asarray(v, dtype=np.float32))
    z = np.ascontiguousarray(np.asarray(z, dtype=np.float32))
    return [
        {"v": v[i * RPC : (i + 1) * RPC], "z": z[i * RPC : (i + 1) * RPC]}
        for i in range(N_CORES)
    ]


def run_spmd(v: np.ndarray, z: np.ndarray, **kwargs):
    """Run on all 8 cores; returns BassKernelResults (kwargs e.g. trace=True)."""
    return run_bass_kernel_spmd(_get_nc(), _in_maps(v, z), list(range(N_CORES)), **kwargs)


def kernel(v: np.ndarray, z: np.ndarray) -> np.ndarray:
    res = run_spmd(v, z)
    return np.concatenate([res.results[i]["out"] for i in range(N_CORES)], axis=0)
